# revision 11
# baseline (speedup 1.0000x reference)
"""Trainium2 Bass kernel for nn_NeuralNet_19516331393457 (dense_mlp).

Pipeline: x = embed[data] (48-entry table); h1 = relu(x@W1+b1);
h2 = tanh(h1@W2+b2); out = h2@W3+b3; return out[argmax(F(out0, out1))].

Strategy (data-parallel over N=500000 on 8 cores):
  - Host: tiny-table gather embed[data] in fp8e4, laid out for fp8
    DoubleRow matmul: per quad [64, (chunk, pairhalf, sample)] so the
    contraction over 128 dims folds to 64 partitions x 2 packed rows.
  - Device, software-pipelined per quad q (4 chunks x 512 samples):
      * MM1 x4 fp8 DoubleRow (0.5 col/cycle) -> 2x [128,1024] PSUM
      * relu evictions split DVE/ACT (PSUM reads are 1x-rate; these
        engines are the kernel's capacity bound)
      * MM2 x4 bf16 -> one [128,1024] PSUM tile (2 chunks stacked/col)
      * one ACT tanh evicts the quad -> h2 fp8 [128, 2, 512]
      * MM3 x1 fp8 DoubleRow with 4-up block-diag W3 -> [8,512] dense
        rows; 4 quads pack one PSUM bank at tile_position col 32p
      * one cast per 4 quads [104,512] -> fp16 staging -> out DMA
      * PSUM budget: p1 2x2 + p2 1x2 + po 2x1 = 8 banks exactly
  - Host: decode outs, F in fp64, exact top-K=4096 rescore in fp32
    (fp8 device math keeps the true winner at rank <=1; rescore
    returns the exact answer).
"""

import numpy as np
import ml_dtypes

import concourse.mybir as mybir
import concourse.tile as tile
from concourse import bacc
from concourse.bass_utils import run_bass_kernel_spmd

N = 500000
D = 128
H1 = 128
H2 = 64
NCLS = 2
NCORES = 8
CHUNK = 512
NPC_RAW = N // NCORES              # 62500 samples per core
NQ = 31                            # quads per core (4 chunks each)
CHUNKS = 4 * NQ                    # 124
NPC = CHUNKS * CHUNK               # 63488 padded samples per core
NG = 16                            # 2-quad po groups (last is half)

_F32 = mybir.dt.float32
_F16 = mybir.dt.float16
_BF16 = mybir.dt.bfloat16
_FP8 = mybir.dt.float8e4
_DR = mybir.MatmulPerfMode.DoubleRow

NP_FP8 = ml_dtypes.float8_e4m3


def _issue_x_dma(nc, q, pools, tls, xts):
    (xpool, h1pool, h2pool, obpool, p1pool, p2pool, popool) = pools
    xt = xpool.tile([D, 2, 2, CHUNK], _FP8, name=f"xt{q}", tag="xt")
    nc.sync.dma_start(xt[:], tls["x_t"][q])
    xts[q] = xt


def _quad_head_mm(nc, q, pools, tls, xts):
    """Issue MM1 for quad q: fp8 DoubleRow at full 128 partitions.

    The moving AP [128, 2, 512] pairs the two chunks of a quad-half;
    the stationary alternates [W1, 0] / [0, W1] so each matmul computes
    one chunk's W1^T x while streaming both (0.5 col/cycle)."""
    (xpool, h1pool, h2pool, obpool, p1pool, p2pool, popool) = pools
    xt = xts.pop(q)

    p1a = p1pool.tile([H1, 2 * CHUNK], _F32, name=f"p1a{q}", tag="p1")
    p1b = p1pool.tile([H1, 2 * CHUNK], _F32, name=f"p1b{q}", tag="p1")
    for g, p1t in ((0, p1a), (1, p1b)):
        for i in range(2):
            nc.tensor.matmul(
                p1t[:, i * CHUNK : (i + 1) * CHUNK],
                tls["w1sb"][:, i],
                xt[:, g],
                start=True, stop=True,
                perf_mode=_DR,
            )
    return p1a, p1b


def _quad_head_relu(nc, q, pools, tls, p1ab):
    """relu+bias evictions; engine split tuned for ACT@1.2 vs DVE@0.96."""
    (xpool, h1pool, h2pool, obpool, p1pool, p2pool, popool) = pools
    p1a, p1b = p1ab
    h1a = h1pool.tile([H1, 2 * CHUNK], _BF16, name=f"h1a{q}", tag="h1")
    nc.vector.tensor_scalar(
        h1a[:], p1a[:], tls["b1sb"], 0.0,
        mybir.AluOpType.add, mybir.AluOpType.max,
    )
    h1b = h1pool.tile([H1, 2 * CHUNK], _BF16, name=f"h1b{q}", tag="h1")
    if q % 8 in (2, 5, 7):
        nc.vector.tensor_scalar(
            h1b[:], p1b[:], tls["b1sb"], 0.0,
            mybir.AluOpType.add, mybir.AluOpType.max,
        )
    else:
        nc.scalar.activation(
            h1b[:], p1b[:], mybir.ActivationFunctionType.Relu,
            bias=tls["b1sb"],
        )
    return h1a, h1b


def _quad_tail_a(nc, q, pools, tls, h1ab):
    """Issue MM2 (bf16) + tanh->fp8 for quad q."""
    (xpool, h1pool, h2pool, obpool, p1pool, p2pool, popool) = pools
    h1a, h1b = h1ab
    p2 = p2pool.tile([128, 2 * CHUNK], _F32, name=f"p2_{q}", tag="p2")
    for h, h1t in ((0, h1a), (1, h1b)):
        for s in range(2):
            nc.tensor.matmul(
                p2[s * H2 : (s + 1) * H2, h * CHUNK : (h + 1) * CHUNK],
                tls["w2sb"],
                h1t[:, s * CHUNK : (s + 1) * CHUNK],
                start=True, stop=True,
            )

    h2t = h2pool.tile([128, 2, CHUNK], _FP8, name=f"h2_{q}", tag="h2")
    nc.scalar.activation(
        h2t[:], p2[:], mybir.ActivationFunctionType.Tanh,
        bias=tls["b2sb"],
    )
    return h2t


def _quad_tail_b(nc, q, pools, tls, h2t, pos, obs):
    """Issue MM3 (fp8 block-diag 2-up) + cast/out per 2-quad group."""
    (xpool, h1pool, h2pool, obpool, p1pool, p2pool, popool) = pools
    # MM3 via block-diag W3blk [128,4]: one matmul per h2 column-pair makes
    # a dense [4,512] output; 4 pairs (2 quads) pack one [*,512] PSUM bank.
    g = q // 2
    if q % 2 == 0:
        pos[g] = popool.tile([128, CHUNK], _F32, name=f"po{g}", tag="po")
    po = pos[g]
    for h in range(2):
        p = 2 * (q % 2) + h
        nc.tensor.matmul(
            po[32 * p : 32 * p + 4, :],
            tls["w3sb"],
            h2t[:, h],
            start=True, stop=True,
            tile_position=(0, 32 * p),
        )

    if q % 2 == 1 or q == NQ - 1:
        f = g // 2
        if q % 4 == 1 and q != NQ - 1:
            obs[f] = obpool.tile([128, 2 * CHUNK], _F16, name=f"ob{f}",
                                 tag="ob")
        ob = obs[f]
        slot = g % 2
        csl = slice(slot * CHUNK, (slot + 1) * CHUNK)
        if g % 4 in (0, 1):
            nc.scalar.activation(
                ob[0:100, csl], po[0:100, :],
                mybir.ActivationFunctionType.Copy,
            )
        else:
            nc.vector.tensor_copy(ob[0:100, csl], po[0:100, :])
        if slot == 1 or q == NQ - 1:
            base = f * 2 * CHUNK
            cols = (slot + 1) * CHUNK
            nc.sync.dma_start(
                tls["out_d"][:, base : base + cols], ob[0:100, 0:cols],
            )


def _build_bass():
    nc = bacc.Bacc(
        "TRN2",
        target_bir_lowering=False,
        debug=False,
        enable_asserts=False,
        num_devices=NCORES,
    )
    x_t = nc.dram_tensor("x_t", [NQ, D, 2, 2, CHUNK], _FP8,
                         kind="ExternalInput")
    # variant v: [W1, 0] (v=0) / [0, W1] (v=1) pair-weights for DoubleRow
    w1pk = nc.dram_tensor("w1pk", [D, 2, 2, H1], _FP8, kind="ExternalInput")
    w2pk = nc.dram_tensor("w2pk", [H1, H2], _BF16, kind="ExternalInput")
    # block-diag W3blk [128, 4] = [W3 0; 0 W3]
    w3pk = nc.dram_tensor("w3pk", [128, 4], _FP8, kind="ExternalInput")
    # packed biases: col 0 = b1, col 1 = [b2; b2]
    bpk = nc.dram_tensor("bpk", [128, 2], _F32, kind="ExternalInput")
    # row 32p+r = class r%2 of in-group chunk 4(p//2)+2(p%2)+r//2, group g
    # at cols [g*512:(g+1)*512] (dense partition dump of the po layout)
    out_d = nc.dram_tensor("out_d", [100, NG * CHUNK], _F16,
                           kind="ExternalOutput")

    with tile.TileContext(nc) as tc:
        with (
            tc.tile_pool(name="w", bufs=1) as wpool,
            tc.tile_pool(name="x", bufs=4) as xpool,
            tc.tile_pool(name="x0", bufs=4) as x0pool,
            tc.tile_pool(name="h1", bufs=4) as h1pool,
            tc.tile_pool(name="h2", bufs=3) as h2pool,
            tc.tile_pool(name="ob", bufs=2) as obpool,
            tc.tile_pool(name="p1", bufs=2, space="PSUM") as p1pool,
            tc.tile_pool(name="p2", bufs=1, space="PSUM") as p2pool,
            tc.tile_pool(name="po", bufs=2, space="PSUM") as popool,
        ):
            tls = {"x_t": x_t, "out_d": out_d}
            pools = (xpool, h1pool, h2pool, obpool, p1pool, p2pool, popool)
            xts = {}

            # quad 0's x first as four per-chunk DMAs (first MM1 waits 64KB)
            x0 = x0pool.tile([D, 2, 2, CHUNK], _FP8, name="x0", tag="x0")
            for g in range(2):
                for i in range(2):
                    nc.sync.dma_start(x0[:, g, i], x_t[0, :, g, i])
            xts[0] = x0
            _issue_x_dma(nc, 1, pools, tls, xts)
            # weights via the GPSIMD (SWDGE) queue, off the x-critical path
            w1sb = wpool.tile([D, 2, 2, H1], _FP8)
            nc.gpsimd.dma_start(w1sb[:], w1pk[:])
            w2sb = wpool.tile([H1, H2], _BF16)
            nc.gpsimd.dma_start(w2sb[:], w2pk[:])
            w3sb = wpool.tile([128, 4], _FP8)
            nc.gpsimd.dma_start(w3sb[:], w3pk[:])
            bsb = wpool.tile([128, 2], _F32)
            nc.gpsimd.dma_start(bsb[:], bpk[:])
            tls.update({
                "w1sb": w1sb[:], "w2sb": w2sb[:], "w3sb": w3sb[:],
                "b1sb": bsb[:, 0:1], "b2sb": bsb[:, 1:2],
            })

            # interleaved issue so each engine queue matches input-readiness:
            # PE: MM1(q), MM2(q-1), MM3(q-1); ACT: tanh(q-1), relu-b(q);
            # DVE: relu-a(q), cast
            pos, obs = {}, {}
            prev_h1 = None
            prev_h2 = None
            for q in range(NQ):
                if q + 2 < NQ:
                    _issue_x_dma(nc, q + 2, pools, tls, xts)
                p1ab = _quad_head_mm(nc, q, pools, tls, xts)
                if prev_h1 is not None:
                    prev_h2 = (q - 1, _quad_tail_a(nc, q - 1, pools, tls,
                                                   prev_h1))
                prev_h1 = _quad_head_relu(nc, q, pools, tls, p1ab)
                if prev_h2 is not None:
                    _quad_tail_b(nc, prev_h2[0], pools, tls, prev_h2[1],
                                 pos, obs)
                    prev_h2 = None
            h2last = _quad_tail_a(nc, NQ - 1, pools, tls, prev_h1)
            _quad_tail_b(nc, NQ - 1, pools, tls, h2last, pos, obs)

    nc.compile()
    return nc


_NC_CACHE = None


def _get_nc():
    global _NC_CACHE
    if _NC_CACHE is None:
        _NC_CACHE = _build_bass()
    return _NC_CACHE


def _F64(x, y):
    return (
        3.0 * (1.0 - x) ** 2 * np.exp(-(x**2) - (y + 1.0) ** 2)
        - 10.0 * (x / 5.0 - x**3 - y**5) * np.exp(-(x**2) - y**2)
        - 1.0 / (3.0 ** np.exp(-((x + 1.0) ** 2) - y**2))
    )


def make_in_maps(data, embed, W1, b1, W2, b2, W3, b3):
    data = np.asarray(data)
    table8 = np.asarray(embed, dtype=np.float32).reshape(-1).astype(NP_FP8)

    W1f = np.asarray(W1, np.float32)
    w1pk = np.zeros((D, 2, 2, H1), np.float32)
    w1pk[:, 0, 0, :] = W1f
    w1pk[:, 1, 1, :] = W1f
    w1pk = np.ascontiguousarray(w1pk.astype(NP_FP8))

    w2pk = np.ascontiguousarray(
        np.asarray(W2, np.float32).astype(ml_dtypes.bfloat16))

    W3f = np.asarray(W3, np.float32)
    w3pk = np.zeros((128, 4), np.float32)
    w3pk[0:64, 0:2] = W3f
    w3pk[64:128, 2:4] = W3f
    w3pk = np.ascontiguousarray(w3pk.astype(NP_FP8))

    b2c = np.asarray(b2, dtype=np.float32).reshape(H2, 1)
    bpk = np.zeros((128, 2), np.float32)
    bpk[:, 0:1] = np.ascontiguousarray(b1, dtype=np.float32).reshape(H1, 1)
    bpk[:, 1:2] = np.concatenate([b2c, b2c], axis=0)

    in_maps = []
    for c in range(NCORES):
        dshard = data[c * NPC_RAW : (c + 1) * NPC_RAW]
        dpad = np.zeros((NPC, D), dtype=dshard.dtype)
        dpad[:NPC_RAW] = dshard
        xt = np.ascontiguousarray(
            table8[dpad.reshape(NQ, 4 * CHUNK, D).transpose(0, 2, 1)]
        ).reshape(NQ, D, 2, 2, CHUNK)
        in_maps.append({"x_t": xt, "w1pk": w1pk, "w2pk": w2pk,
                       "w3pk": w3pk, "bpk": bpk})
    return in_maps


def _decode_outs(res):
    """-> out0_all, out1_all fp32 arrays of shape [N] (padding stripped)."""
    o0s, o1s = [], []
    for c in range(NCORES):
        od = np.asarray(res.results[c]["out_d"], np.float32)
        arr = od.reshape(100, NG, CHUNK)            # [row, g, i]
        o0 = np.empty((CHUNKS, CHUNK), np.float32)
        o1 = np.empty((CHUNKS, CHUNK), np.float32)
        for r in range(4):
            for p in range(4):
                ch = 4 * (p // 2) + 2 * (p % 2) + (r // 2)  # chunk-in-group
                dst = o0 if r % 2 == 0 else o1
                ks = np.arange(NG) * 8 + ch
                valid = ks < CHUNKS
                dst[ks[valid]] = arr[32 * p + r, valid]
        o0s.append(o0.reshape(-1)[:NPC_RAW])
        o1s.append(o1.reshape(-1)[:NPC_RAW])
    return np.concatenate(o0s), np.concatenate(o1s)


def kernel(data, embed, W1, b1, W2, b2, W3, b3):
    data = np.asarray(data)
    nc = _get_nc()
    in_maps = make_in_maps(data, embed, W1, b1, W2, b2, W3, b3)
    res = run_bass_kernel_spmd(nc, in_maps, core_ids=list(range(NCORES)))
    o0, o1 = _decode_outs(res)

    pred = _F64(o0.astype(np.float64), o1.astype(np.float64))
    K = 4096
    cand = np.argpartition(pred, N - K)[N - K:]

    table32 = np.asarray(embed, dtype=np.float32).reshape(-1)
    W1f = np.asarray(W1, np.float32)
    W2f = np.asarray(W2, np.float32)
    W3f = np.asarray(W3, np.float32)
    xk = table32[data[cand]]
    hk = np.maximum(xk @ W1f + np.asarray(b1, np.float32), 0.0)
    hk = np.tanh(hk @ W2f + np.asarray(b2, np.float32))
    ok = hk @ W3f + np.asarray(b3, np.float32)
    pk = _F64(ok[:, 0].astype(np.float64), ok[:, 1].astype(np.float64))
    return ok[int(np.argmax(pk))].astype(np.float32)


# revision 12
# speedup vs baseline: 1.0565x; 1.0565x over previous
"""Trainium2 Bass kernel for nn_NeuralNet_19516331393457 (dense_mlp).

Pipeline: x = embed[data] (48-entry table); h1 = relu(x@W1+b1);
h2 = tanh(h1@W2+b2); out = h2@W3+b3; return out[argmax(F(out0, out1))].

Strategy (data-parallel over N=500000 on 8 cores, fp8 device math):
  - Host: tiny-table gather embed[data] in fp8e4 (halves input DMA to
    8.1MB/core), tile-blocked transpose to [NQ, 128, 2048] per core.
  - Device, software-pipelined per quad q (4 chunks x 512 samples),
    chunk-granular PSUM so buffer recycling never stalls the PE:
      * MM1 x4 fp8 -> four [128,512] PSUM banks (one per chunk)
      * relu evictions [128,512] split DVE/ACT per-quad-balanced
        (PSUM reads are 1x-rate; these engines are the capacity bound)
      * MM2 x4 bf16 -> two [128,512] PSUM banks (chunk pair stacked)
      * tanh x2 [128,512] on ACT -> h2 fp8
      * MM3 fp8 block-diag W3blk 2-up -> [4,512] dense rows; 4 pairs
        (2 quads) pack one PSUM bank at tile_position col 32p
      * cast [100,512] per 2 quads -> fp16 staging -> out DMA per group
      * PSUM budget: p1 4x1 + p2 2x1 + po 2x1 = 8 banks exactly
  - Host: decode outs, F in fp64, exact top-K=4096 rescore in fp32
    (fp8 keeps the true winner at rank <=1; rescore is exact).
"""

import numpy as np
import ml_dtypes

import concourse.mybir as mybir
import concourse.tile as tile
from concourse import bacc
from concourse.bass_utils import run_bass_kernel_spmd

N = 500000
D = 128
H1 = 128
H2 = 64
NCLS = 2
NCORES = 8
CHUNK = 512
NPC_RAW = N // NCORES              # 62500 samples per core
NQ = 31                            # quads per core (4 chunks each)
CHUNKS = 4 * NQ                    # 124
NPC = CHUNKS * CHUNK               # 63488 padded samples per core
NG = 16                            # 2-quad po groups (last is half)

_F32 = mybir.dt.float32
_F16 = mybir.dt.float16
_BF16 = mybir.dt.bfloat16
_FP8 = mybir.dt.float8e4

NP_FP8 = ml_dtypes.float8_e4m3


def _issue_x_dma(nc, q, pools, tls, xts):
    (xpool, h1pool, h2pool, obpool, p1pool, p2pool, popool) = pools
    xt = xpool.tile([D, 4 * CHUNK], _FP8, name=f"xt{q}", tag="xt")
    nc.sync.dma_start(xt[:], tls["x_t"][q])
    xts[q] = xt


def _quad_head_mm(nc, q, pools, tls, xts):
    """Issue MM1 (fp8) for quad q: one [128,512] PSUM bank per chunk."""
    (xpool, h1pool, h2pool, obpool, p1pool, p2pool, popool) = pools
    xt = xts.pop(q)
    p1s = []
    for c in range(4):
        p1 = p1pool.tile([H1, CHUNK], _F32, name=f"p1_{q}_{c}", tag="p1")
        nc.tensor.matmul(
            p1[:], tls["w1sb"], xt[:, c * CHUNK : (c + 1) * CHUNK],
            start=True, stop=True,
        )
        p1s.append(p1)
    return p1s


def _relu_one(nc, eng, q, c, pools, tls, p1):
    (xpool, h1pool, h2pool, obpool, p1pool, p2pool, popool) = pools
    h1 = h1pool.tile([H1, CHUNK], _BF16, name=f"h1_{q}_{c}", tag="h1")
    if eng == "dve":
        nc.vector.tensor_scalar(
            h1[:], p1[:], tls["b1sb"], 0.0,
            mybir.AluOpType.add, mybir.AluOpType.max,
        )
    else:
        nc.scalar.activation(
            h1[:], p1[:], mybir.ActivationFunctionType.Relu,
            bias=tls["b1sb"],
        )
    return h1


def _quad_head_relu(nc, q, pools, tls, p1s):
    """relu+bias evictions, chunk-granular; per-quad-balanced split.

    Per 2 quads: ACT gets 4 tanh + 3 relu, DVE gets 5 relu + 1 cast,
    matching ACT@1.2GHz vs DVE@0.96GHz rates."""
    act_chunks = (2,) if q % 2 == 0 else (2, 3)
    h1s = []
    for c in range(4):
        eng = "act" if c in act_chunks else "dve"
        h1s.append(_relu_one(nc, eng, q, c, pools, tls, p1s[c]))
    return h1s


def _quad_tail_a(nc, q, pools, tls, h1s):
    """Issue MM2 (bf16) + tanh->fp8 for quad q, half-granular."""
    (xpool, h1pool, h2pool, obpool, p1pool, p2pool, popool) = pools
    h2s = []
    for h in range(2):
        p2 = p2pool.tile([128, CHUNK], _F32, name=f"p2_{q}_{h}", tag="p2")
        for s in range(2):
            nc.tensor.matmul(
                p2[s * H2 : (s + 1) * H2, :],
                tls["w2sb"],
                h1s[2 * h + s][:],
                start=True, stop=True,
            )
        h2 = h2pool.tile([128, CHUNK], _FP8, name=f"h2_{q}_{h}", tag="h2")
        nc.scalar.activation(
            h2[:], p2[:], mybir.ActivationFunctionType.Tanh,
            bias=tls["b2sb"],
        )
        h2s.append(h2)
    return h2s


def _quad_tail_b(nc, q, pools, tls, h2s, pos, obs):
    """Issue MM3 (fp8 block-diag 2-up) + cast + out DMA per group."""
    (xpool, h1pool, h2pool, obpool, p1pool, p2pool, popool) = pools
    # MM3 via block-diag W3blk [128,4]: one matmul per h2 half makes a
    # dense [4,512] output; 4 halves (2 quads) pack one [*,512] PSUM bank.
    g = q // 2
    if q % 2 == 0:
        pos[g] = popool.tile([128, CHUNK], _F32, name=f"po{g}", tag="po")
    po = pos[g]
    for h in range(2):
        p = 2 * (q % 2) + h
        nc.tensor.matmul(
            po[32 * p : 32 * p + 4, :],
            tls["w3sb"],
            h2s[h][:],
            start=True, stop=True,
            tile_position=(0, 32 * p),
        )

    if q % 2 == 1 or q == NQ - 1:
        ob = obpool.tile([128, CHUNK], _F16, name=f"ob{g}", tag="ob")
        nc.vector.tensor_copy(ob[0:100, :], po[0:100, :])
        nc.sync.dma_start(
            tls["out_d"][:, g * CHUNK : (g + 1) * CHUNK], ob[0:100, :],
        )


def _build_bass():
    nc = bacc.Bacc(
        "TRN2",
        target_bir_lowering=False,
        debug=False,
        enable_asserts=False,
        num_devices=NCORES,
    )
    x_t = nc.dram_tensor("x_t", [NQ, D, 4 * CHUNK], _FP8,
                         kind="ExternalInput")
    w1pk = nc.dram_tensor("w1pk", [D, H1], _FP8, kind="ExternalInput")
    w2pk = nc.dram_tensor("w2pk", [H1, H2], _BF16, kind="ExternalInput")
    # block-diag W3blk [128, 4] = [W3 0; 0 W3]
    w3pk = nc.dram_tensor("w3pk", [128, 4], _FP8, kind="ExternalInput")
    # packed biases: col 0 = b1, col 1 = [b2; b2]
    bpk = nc.dram_tensor("bpk", [128, 2], _F32, kind="ExternalInput")
    # row 32p+r = class r%2 of in-group chunk 4(p//2)+2(p%2)+r//2, group g
    # at cols [g*512:(g+1)*512] (dense partition dump of the po layout)
    out_d = nc.dram_tensor("out_d", [100, NG * CHUNK], _F16,
                           kind="ExternalOutput")

    with tile.TileContext(nc) as tc:
        with (
            tc.tile_pool(name="w", bufs=1) as wpool,
            tc.tile_pool(name="x", bufs=3) as xpool,
            tc.tile_pool(name="h1", bufs=8) as h1pool,
            tc.tile_pool(name="h2", bufs=5) as h2pool,
            tc.tile_pool(name="ob", bufs=2) as obpool,
            tc.tile_pool(name="p1", bufs=4, space="PSUM") as p1pool,
            tc.tile_pool(name="p2", bufs=2, space="PSUM") as p2pool,
            tc.tile_pool(name="po", bufs=2, space="PSUM") as popool,
        ):
            tls = {"x_t": x_t, "out_d": out_d}
            pools = (xpool, h1pool, h2pool, obpool, p1pool, p2pool, popool)
            xts = {}

            # w1 first on the sync queue (it gates the first MM1), then
            # quad 0's x as a single DMA
            w1sb = wpool.tile([D, H1], _FP8)
            nc.sync.dma_start(w1sb[:], w1pk[:])
            _issue_x_dma(nc, 0, pools, tls, xts)
            _issue_x_dma(nc, 1, pools, tls, xts)
            # remaining weights via the GPSIMD (SWDGE) queue
            w2sb = wpool.tile([H1, H2], _BF16)
            nc.gpsimd.dma_start(w2sb[:], w2pk[:])
            w3sb = wpool.tile([128, 4], _FP8)
            nc.gpsimd.dma_start(w3sb[:], w3pk[:])
            bsb = wpool.tile([128, 2], _F32)
            nc.gpsimd.dma_start(bsb[:], bpk[:])
            tls.update({
                "w1sb": w1sb[:], "w2sb": w2sb[:], "w3sb": w3sb[:],
                "b1sb": bsb[:, 0:1], "b2sb": bsb[:, 1:2],
            })

            # preload the ACT function table off the critical path: a tiny
            # tanh on a memset scratch while the first x DMA is in flight
            scr = wpool.tile([128, 4], _F32)
            nc.gpsimd.memset(scr[:], 0.0)
            scr2 = wpool.tile([128, 4], _F32)
            nc.scalar.activation(scr2[:], scr[:],
                                 mybir.ActivationFunctionType.Tanh)

            # interleaved issue so each engine queue matches input-readiness:
            # PE: MM1(q), MM2(q-1), MM3(q-1); ACT: relu-share(q), tanh(q-1);
            # DVE: relu-share(q), cast(q-1)
            pos, obs = {}, {}
            prev_h1 = None
            prev_h2 = None
            for q in range(NQ):
                if q + 2 < NQ:
                    _issue_x_dma(nc, q + 2, pools, tls, xts)
                p1s = _quad_head_mm(nc, q, pools, tls, xts)
                h1s = _quad_head_relu(nc, q, pools, tls, p1s)
                if prev_h1 is not None:
                    prev_h2 = (q - 1, _quad_tail_a(nc, q - 1, pools, tls,
                                                   prev_h1))
                prev_h1 = h1s
                if prev_h2 is not None:
                    _quad_tail_b(nc, prev_h2[0], pools, tls, prev_h2[1],
                                 pos, obs)
                    prev_h2 = None
            h2last = _quad_tail_a(nc, NQ - 1, pools, tls, prev_h1)
            _quad_tail_b(nc, NQ - 1, pools, tls, h2last, pos, obs)

    nc.compile()
    return nc


_NC_CACHE = None


def _get_nc():
    global _NC_CACHE
    if _NC_CACHE is None:
        _NC_CACHE = _build_bass()
    return _NC_CACHE


def _F64(x, y):
    return (
        3.0 * (1.0 - x) ** 2 * np.exp(-(x**2) - (y + 1.0) ** 2)
        - 10.0 * (x / 5.0 - x**3 - y**5) * np.exp(-(x**2) - y**2)
        - 1.0 / (3.0 ** np.exp(-((x + 1.0) ** 2) - y**2))
    )


def make_in_maps(data, embed, W1, b1, W2, b2, W3, b3):
    data = np.asarray(data)
    table8 = np.asarray(embed, dtype=np.float32).reshape(-1).astype(NP_FP8)

    w1pk = np.ascontiguousarray(np.asarray(W1, np.float32).astype(NP_FP8))
    w2pk = np.ascontiguousarray(
        np.asarray(W2, np.float32).astype(ml_dtypes.bfloat16))

    W3f = np.asarray(W3, np.float32)
    w3pk = np.zeros((128, 4), np.float32)
    w3pk[0:64, 0:2] = W3f
    w3pk[64:128, 2:4] = W3f
    w3pk = np.ascontiguousarray(w3pk.astype(NP_FP8))

    b2c = np.asarray(b2, dtype=np.float32).reshape(H2, 1)
    bpk = np.zeros((128, 2), np.float32)
    bpk[:, 0:1] = np.ascontiguousarray(b1, dtype=np.float32).reshape(H1, 1)
    bpk[:, 1:2] = np.concatenate([b2c, b2c], axis=0)

    in_maps = []
    for c in range(NCORES):
        dshard = data[c * NPC_RAW : (c + 1) * NPC_RAW]
        dpad = np.zeros((NPC, D), dtype=dshard.dtype)
        dpad[:NPC_RAW] = dshard
        xt = np.ascontiguousarray(
            table8[dpad.reshape(NQ, 4 * CHUNK, D).transpose(0, 2, 1)]
        )
        in_maps.append({"x_t": xt, "w1pk": w1pk, "w2pk": w2pk,
                       "w3pk": w3pk, "bpk": bpk})
    return in_maps


def _decode_outs(res):
    """-> out0_all, out1_all fp32 arrays of shape [N] (padding stripped)."""
    o0s, o1s = [], []
    for c in range(NCORES):
        od = np.asarray(res.results[c]["out_d"], np.float32)
        arr = od.reshape(100, NG, CHUNK)            # [row, g, i]
        o0 = np.empty((CHUNKS, CHUNK), np.float32)
        o1 = np.empty((CHUNKS, CHUNK), np.float32)
        for r in range(4):
            for p in range(4):
                ch = 4 * (p // 2) + 2 * (p % 2) + (r // 2)  # chunk-in-group
                dst = o0 if r % 2 == 0 else o1
                ks = np.arange(NG) * 8 + ch
                valid = ks < CHUNKS
                dst[ks[valid]] = arr[32 * p + r, valid]
        o0s.append(o0.reshape(-1)[:NPC_RAW])
        o1s.append(o1.reshape(-1)[:NPC_RAW])
    return np.concatenate(o0s), np.concatenate(o1s)


def kernel(data, embed, W1, b1, W2, b2, W3, b3):
    data = np.asarray(data)
    nc = _get_nc()
    in_maps = make_in_maps(data, embed, W1, b1, W2, b2, W3, b3)
    res = run_bass_kernel_spmd(nc, in_maps, core_ids=list(range(NCORES)))
    o0, o1 = _decode_outs(res)

    pred = _F64(o0.astype(np.float64), o1.astype(np.float64))
    K = 4096
    cand = np.argpartition(pred, N - K)[N - K:]

    table32 = np.asarray(embed, dtype=np.float32).reshape(-1)
    W1f = np.asarray(W1, np.float32)
    W2f = np.asarray(W2, np.float32)
    W3f = np.asarray(W3, np.float32)
    xk = table32[data[cand]]
    hk = np.maximum(xk @ W1f + np.asarray(b1, np.float32), 0.0)
    hk = np.tanh(hk @ W2f + np.asarray(b2, np.float32))
    ok = hk @ W3f + np.asarray(b3, np.float32)
    pk = _F64(ok[:, 0].astype(np.float64), ok[:, 1].astype(np.float64))
    return ok[int(np.argmax(pk))].astype(np.float32)
